# revision 1
# baseline (speedup 1.0000x reference)
"""BDH dense-transformer Trainium2 kernel (8 NeuronCores, SPMD).

Model (weight-tied, 4 layers): T=1024, D=256, NH=4, N=8192/head, VOCAB=256.

Sharding: core c -> head h=c//2, latent half j=c%2 (4096 latent dims/core).
  - encoder/encoder_v column-sharded, decoder row-sharded (host-permuted so
    rope pairs are de-interleaved: local m in [0,2048) = even pair elements,
    [2048,4096) = odd elements; permutation applied consistently to all three
    weight shards so scores/gate/decoder are unaffected).
  - scores trick: yKV = mask(qr qr^T) @ x distributes over latent shards:
    each core computes mask(qr_loc qr_loc^T) @ x, pairwise AllReduce on yKV.
  - decoder partial sums: 8-way AllReduce on y.

On-device layouts: big tensors feature-major [latent, T] (contraction dims on
partitions); residual stream x token-major [T, D] fp32; matmuls bf16 with
fp32 PSUM accumulation.
"""

import os
import numpy as np
import ml_dtypes

BF16NP = ml_dtypes.bfloat16

# full-size config
T = 1024
D = 256
NH = 4
N = 8192
V = 256
L = 4
NCORES = 8
P = 128
EPS = 1e-5
THETA = 2.0 ** 16

_CACHE = {}


def _cfg(n_cores=NCORES, half=N // 2, t=T, layers=L):
    # half: latent width per core (= N*NH/n_cores)
    assert t % 512 == 0 and half % 256 == 0
    return dict(
        n_cores=n_cores, half=half, t=t, layers=layers,
        tch=t // P,            # token chunks
        kch=half // P,         # latent chunks
        pblk=half // 2 // P,   # pair blocks (even/odd chunk pairs)
        tb_n=t // 512,         # 512-wide t blocks
        dch=D // P,            # 2
    )


# ---------------------------------------------------------------- device code

def emit_model(tc, in_aps, out_ap, cfg):
    from contextlib import ExitStack
    import concourse.mybir as mybir
    from concourse.masks import make_identity

    nc = tc.nc
    f32 = mybir.dt.float32
    bf = mybir.dt.bfloat16
    ADD = mybir.AluOpType.add
    SUB = mybir.AluOpType.subtract
    MULT = mybir.AluOpType.mult
    Relu = mybir.ActivationFunctionType.Relu
    Sqrt = mybir.ActivationFunctionType.Sqrt

    n_cores = cfg["n_cores"]
    TCH, KCH, PBLK, TB, DCH = (cfg[k] for k in ("tch", "kch", "pblk", "tb_n", "dch"))
    TT = cfg["t"]
    HALF = cfg["half"]
    layers = cfg["layers"]
    pair_groups = [[2 * i, 2 * i + 1] for i in range(n_cores // 2)]
    all_group = [list(range(n_cores))]

    with ExitStack() as ctx:
        singles = ctx.enter_context(tc.tile_pool(name="singles", bufs=1))
        state = ctx.enter_context(tc.tile_pool(name="state", bufs=1))
        dram = ctx.enter_context(tc.tile_pool(name="dramp", bufs=1, space="DRAM"))
        lnp = ctx.enter_context(tc.tile_pool(name="lnp", bufs=4))

        # ---- resident constants
        we_sb = singles.tile([P, DCH, HALF], bf, name="we_sb")
        nc.sync.dma_start(out=we_sb[:], in_=in_aps["w_e"].rearrange("(ko p) m -> p ko m", p=P))
        wv_sb = singles.tile([P, DCH, HALF], bf, name="wv_sb")
        nc.sync.dma_start(out=wv_sb[:], in_=in_aps["w_v"].rearrange("(ko p) m -> p ko m", p=P))
        dec_sb = singles.tile([P, KCH, D], bf, name="dec_sb")
        nc.sync.dma_start(out=dec_sb[:], in_=in_aps["dec"].rearrange("(kc p) d -> p kc d", p=P))
        lm_sb = singles.tile([P, DCH, V], bf, name="lm_sb")
        nc.sync.dma_start(out=lm_sb[:], in_=in_aps["lm"].rearrange("(ko p) v -> p ko v", p=P))
        mask_sb = singles.tile([P, P], f32, name="mask_sb")
        nc.sync.dma_start(out=mask_sb[:], in_=in_aps["mask"][:])
        eps_sb = singles.tile([P, 1], f32, name="eps_sb")
        nc.vector.memset(eps_sb[:], EPS)
        id_bf = singles.tile([P, P], bf, name="id_bf")
        make_identity(nc, id_bf[:])
        id_f32 = singles.tile([P, P], f32, name="id_f32")
        make_identity(nc, id_f32[:])

        # ---- resident state
        x_sb = state.tile([P, TCH, D], f32, name="x_sb")        # residual, token-major
        tmaj_bf = state.tile([P, TCH, D], bf, name="tmaj_bf")   # xb / yKV_ln
        dmaj_bf = state.tile([P, DCH, TT], bf, name="dmaj_bf")  # xT / yKV_lnT
        qr_sb = state.tile([P, KCH, TT], bf, name="qr_sb")
        td_f32 = state.tile([P, TCH, D], f32, name="td_f32")    # x0 / yKV / y (token-major)
        yT_sb = state.tile([P, DCH, TT], f32, name="yT_sb")     # y feature-major

        # ---- dram scratch
        xsp_dram = dram.tile([P, KCH, TT], bf, name="xsp_dram")
        ar1_in = dram.tile([P, TCH, D], f32, name="ar1_in")
        ar1_out = dram.tile([P, TCH, D], f32, name="ar1_out")
        ar2_in = dram.tile([P, DCH, TT], f32, name="ar2_in")
        ar2_out = dram.tile([P, DCH, TT], f32, name="ar2_out")

        def emit_ln(src3, dst3):
            # LayerNorm over D for each token chunk; src/dst [P, TCH, D]
            for tau in range(TCH):
                stats = lnp.tile([P, 6], f32, name="ln_stats", tag="ln_stats")
                nc.vector.bn_stats(out=stats[:], in_=src3[:, tau, :])
                mv = lnp.tile([P, 2], f32, name="ln_mv", tag="ln_mv")
                nc.vector.bn_aggr(out=mv[:], in_=stats[:])
                std = lnp.tile([P, 1], f32, name="ln_std", tag="ln_std")
                nc.scalar.activation(out=std[:], in_=mv[:, 1:2], func=Sqrt,
                                     bias=eps_sb[:, 0:1])
                rstd = lnp.tile([P, 1], f32, name="ln_rstd", tag="ln_rstd")
                nc.vector.reciprocal(out=rstd[:], in_=std[:])
                nc.vector.tensor_scalar(out=dst3[:, tau, :], in0=src3[:, tau, :],
                                        scalar1=mv[:, 0:1], scalar2=rstd[:],
                                        op0=SUB, op1=MULT)

        def emit_transpose(ps_pool, src2, dst2, ident, dtype, tag):
            pt = ps_pool.tile([P, P], dtype, name=f"tp_{tag}", tag=f"tp_{tag}")
            nc.tensor.transpose(pt[:], src2, ident[:])
            nc.vector.tensor_copy(out=dst2, in_=pt[:])

        def phase_A(tpps):
            # xb = bf16(x); xT = transpose(xb)
            nc.vector.tensor_copy(out=tmaj_bf[:], in_=x_sb[:])
            for tau in range(TCH):
                for ko in range(DCH):
                    emit_transpose(tpps, tmaj_bf[:, tau, ko * P:(ko + 1) * P],
                                   dmaj_bf[:, ko, tau * P:(tau + 1) * P], id_bf, bf, "a")

        # ---- initial: x = LN(x0)
        nc.sync.dma_start(out=td_f32[:], in_=in_aps["x0"].rearrange("(tau p) d -> p tau d", p=P))
        emit_ln(td_f32, x_sb)

        for layer in range(layers):
            # ---------------- phase A: xb, xT
            with tc.tile_pool(name=f"tpA{layer}", bufs=2, space="PSUM") as tpps:
                phase_A(tpps)

            # ---------------- phase B: x_sp = relu(W_e^T xT); rope -> qr; spill x_sp
            with tc.tile_pool(name=f"pb{layer}", bufs=2) as work, \
                 tc.tile_pool(name=f"pbps{layer}", bufs=4, space="PSUM") as ps:
                for blk in range(PBLK):
                    cos_sb = work.tile([P, TT], bf, name="cos_sb", tag="cos")
                    nc.sync.dma_start(out=cos_sb[:], in_=in_aps["cos_t"][blk * P:(blk + 1) * P, :])
                    sin_sb = work.tile([P, TT], bf, name="sin_sb", tag="sin")
                    nc.sync.dma_start(out=sin_sb[:], in_=in_aps["sin_t"][blk * P:(blk + 1) * P, :])
                    ve = work.tile([P, TT], bf, name="ve", tag="ve")
                    vo = work.tile([P, TT], bf, name="vo", tag="vo")
                    for parity, vt in ((0, ve), (1, vo)):
                        kc = blk + PBLK * parity
                        for tb in range(TB):
                            pt = ps.tile([P, 512], f32, name="xsp_ps", tag="xsp_ps")
                            for ko in range(DCH):
                                nc.tensor.matmul(
                                    pt[:], lhsT=we_sb[:, ko, kc * P:(kc + 1) * P],
                                    rhs=dmaj_bf[:, ko, tb * 512:(tb + 1) * 512],
                                    start=(ko == 0), stop=(ko == DCH - 1))
                            nc.scalar.activation(out=vt[:, tb * 512:(tb + 1) * 512],
                                                 in_=pt[:], func=Relu)
                    nc.sync.dma_start(out=xsp_dram[:, blk, :], in_=ve[:])
                    nc.sync.dma_start(out=xsp_dram[:, blk + PBLK, :], in_=vo[:])
                    t1 = work.tile([P, TT], bf, name="t1", tag="t1")
                    t2 = work.tile([P, TT], bf, name="t2", tag="t2")
                    nc.vector.tensor_mul(out=t1[:], in0=ve[:], in1=cos_sb[:])
                    nc.vector.tensor_mul(out=t2[:], in0=vo[:], in1=sin_sb[:])
                    nc.vector.tensor_tensor(qr_sb[:, blk, :], t1[:], t2[:], SUB)
                    t3 = work.tile([P, TT], bf, name="t3", tag="t1")
                    t4 = work.tile([P, TT], bf, name="t4", tag="t2")
                    nc.vector.tensor_mul(out=t3[:], in0=vo[:], in1=cos_sb[:])
                    nc.vector.tensor_mul(out=t4[:], in0=ve[:], in1=sin_sb[:])
                    nc.vector.tensor_tensor(qr_sb[:, blk + PBLK, :], t3[:], t4[:], ADD)

            # ---------------- phase C: scoresT tiles + yKV partial
            with tc.tile_pool(name=f"pc{layer}", bufs=2) as work, \
                 tc.tile_pool(name=f"pcs{layer}", bufs=2, space="PSUM") as scps, \
                 tc.tile_pool(name=f"pcy{layer}", bufs=2, space="PSUM") as ykps:
                for tb in range(TB):
                    sc_sb = work.tile([P, 4 * (tb + 1), 512], bf, name="sc_sb",
                                      tag=f"sc{tb}", bufs=1)
                    n_s = 4 * (tb + 1)
                    for si in range(n_s):
                        pt = scps.tile([P, 512], f32, name="sc_ps", tag="sc_ps")
                        for kc in range(KCH):
                            nc.tensor.matmul(
                                pt[:], lhsT=qr_sb[:, kc, si * P:(si + 1) * P],
                                rhs=qr_sb[:, kc, tb * 512:(tb + 1) * 512],
                                start=(kc == 0), stop=(kc == KCH - 1))
                        sub_d = si - 4 * tb
                        if 0 <= sub_d <= 3:
                            nc.vector.tensor_mul(
                                out=sc_sb[:, si, sub_d * P:(sub_d + 1) * P],
                                in0=pt[:, sub_d * P:(sub_d + 1) * P], in1=mask_sb[:])
                            if sub_d < 3:
                                nc.vector.tensor_copy(
                                    out=sc_sb[:, si, (sub_d + 1) * P:],
                                    in_=pt[:, (sub_d + 1) * P:])
                        else:
                            nc.vector.tensor_copy(out=sc_sb[:, si, :], in_=pt[:])
                    for sub in range(4):
                        tau = tb * 4 + sub
                        yk = ykps.tile([P, D], f32, name="yk_ps", tag="yk_ps")
                        for si in range(tau + 1):
                            nc.tensor.matmul(
                                yk[:], lhsT=sc_sb[:, si, sub * P:(sub + 1) * P],
                                rhs=tmaj_bf[:, si, :],
                                start=(si == 0), stop=(si == tau))
                        nc.vector.tensor_copy(out=td_f32[:, tau, :], in_=yk[:])

            # ---------------- phase D: AllReduce yKV within head pair
            nc.sync.dma_start(out=ar1_in[:], in_=td_f32[:])
            nc.gpsimd.collective_compute(
                "AllReduce", ADD, replica_groups=pair_groups,
                ins=[ar1_in.opt()], outs=[ar1_out.opt()])
            nc.sync.dma_start(out=td_f32[:], in_=ar1_out[:])

            # ---------------- phase E: LN(yKV) -> bf16, transpose
            with tc.tile_pool(name=f"tpE{layer}", bufs=2, space="PSUM") as tpps:
                emit_ln(td_f32, tmaj_bf)
                for tau in range(TCH):
                    for ko in range(DCH):
                        emit_transpose(tpps, tmaj_bf[:, tau, ko * P:(ko + 1) * P],
                                       dmaj_bf[:, ko, tau * P:(tau + 1) * P], id_bf, bf, "e")

            # ---------------- phase F: y_sp, gate, decoder partials
            with tc.tile_pool(name=f"pf{layer}", bufs=2) as work, \
                 tc.tile_pool(name=f"pfy{layer}", bufs=1, space="PSUM") as psY, \
                 tc.tile_pool(name=f"pfs{layer}", bufs=4, space="PSUM") as psF:
                yT_ps = [psY.tile([P, 512], f32, name=f"yt_ps{i}", tag=f"yt_ps{i}")
                         for i in range(DCH * TB)]
                for blk in range(PBLK):
                    for parity in (0, 1):
                        kc = blk + PBLK * parity
                        ysp = work.tile([P, TT], bf, name="ysp", tag="ysp")
                        for tb in range(TB):
                            pt = psF.tile([P, 512], f32, name="ysp_ps", tag="ysp_ps")
                            for ko in range(DCH):
                                nc.tensor.matmul(
                                    pt[:], lhsT=wv_sb[:, ko, kc * P:(kc + 1) * P],
                                    rhs=dmaj_bf[:, ko, tb * 512:(tb + 1) * 512],
                                    start=(ko == 0), stop=(ko == DCH - 1))
                            nc.scalar.activation(out=ysp[:, tb * 512:(tb + 1) * 512],
                                                 in_=pt[:], func=Relu)
                        xsp = work.tile([P, TT], bf, name="xsp_r", tag="xsp_r")
                        nc.sync.dma_start(out=xsp[:], in_=xsp_dram[:, kc, :])
                        xy = work.tile([P, TT], bf, name="xy", tag="xy")
                        nc.vector.tensor_mul(out=xy[:], in0=xsp[:], in1=ysp[:])
                        first = (blk == 0 and parity == 0)
                        last = (blk == PBLK - 1 and parity == 1)
                        for dh in range(DCH):
                            for tb in range(TB):
                                nc.tensor.matmul(
                                    yT_ps[dh * TB + tb][:],
                                    lhsT=dec_sb[:, kc, dh * P:(dh + 1) * P],
                                    rhs=xy[:, tb * 512:(tb + 1) * 512],
                                    start=first, stop=last, skip_group_check=True)
                for dh in range(DCH):
                    for tb in range(TB):
                        nc.vector.tensor_copy(
                            out=yT_sb[:, dh, tb * 512:(tb + 1) * 512],
                            in_=yT_ps[dh * TB + tb][:])

            # ---------------- phase G: AllReduce y (8 cores), transpose back
            nc.sync.dma_start(out=ar2_in[:], in_=yT_sb[:])
            nc.gpsimd.collective_compute(
                "AllReduce", ADD, replica_groups=all_group,
                ins=[ar2_in.opt()], outs=[ar2_out.opt()])
            nc.sync.dma_start(out=yT_sb[:], in_=ar2_out[:])
            with tc.tile_pool(name=f"tpG{layer}", bufs=2, space="PSUM") as tpps:
                for tau in range(TCH):
                    for ko in range(DCH):
                        emit_transpose(tpps, yT_sb[:, ko, tau * P:(tau + 1) * P],
                                       td_f32[:, tau, ko * P:(ko + 1) * P], id_f32, f32, "g")

            # ---------------- phase H: x = LN(x + LN(y))
            emit_ln(td_f32, td_f32)
            nc.vector.tensor_tensor(x_sb[:], x_sb[:], td_f32[:], ADD)
            emit_ln(x_sb, x_sb)

        # ---------------- logits
        with tc.tile_pool(name="tpZ", bufs=2, space="PSUM") as tpps:
            phase_A(tpps)
        with tc.tile_pool(name="lg", bufs=2) as work, \
             tc.tile_pool(name="lgps", bufs=2, space="PSUM") as ps:
            out_r = out_ap.rearrange("(tau p) v -> p tau v", p=P)
            for tau in range(TCH):
                pt = ps.tile([P, V], f32, name="lg_ps", tag="lg_ps")
                for ko in range(DCH):
                    nc.tensor.matmul(pt[:], lhsT=dmaj_bf[:, ko, tau * P:(tau + 1) * P],
                                     rhs=lm_sb[:, ko, :], start=(ko == 0), stop=(ko == DCH - 1))
                lg = work.tile([P, V], f32, name="lg_sb", tag="lg_sb")
                nc.vector.tensor_copy(out=lg[:], in_=pt[:])
                nc.sync.dma_start(out=out_r[:, tau, :], in_=lg[:])


def build(cfg):
    import concourse.bacc as bacc
    import concourse.tile as tile
    import concourse.mybir as mybir

    f32 = mybir.dt.float32
    bf = mybir.dt.bfloat16
    nc = bacc.Bacc("TRN2", target_bir_lowering=False, debug=False,
                   enable_asserts=False, num_devices=cfg["n_cores"])
    TT, HALF = cfg["t"], cfg["half"]
    in_aps = {
        "x0": nc.dram_tensor("x0", [TT, D], f32, kind="ExternalInput").ap(),
        "w_e": nc.dram_tensor("w_e", [D, HALF], bf, kind="ExternalInput").ap(),
        "w_v": nc.dram_tensor("w_v", [D, HALF], bf, kind="ExternalInput").ap(),
        "dec": nc.dram_tensor("dec", [HALF, D], bf, kind="ExternalInput").ap(),
        "lm": nc.dram_tensor("lm", [D, V], bf, kind="ExternalInput").ap(),
        "cos_t": nc.dram_tensor("cos_t", [HALF // 2, TT], bf, kind="ExternalInput").ap(),
        "sin_t": nc.dram_tensor("sin_t", [HALF // 2, TT], bf, kind="ExternalInput").ap(),
        "mask": nc.dram_tensor("mask", [P, P], f32, kind="ExternalInput").ap(),
    }
    out_ap = nc.dram_tensor("logits", [TT, V], f32, kind="ExternalOutput").ap()
    with tile.TileContext(nc) as tc:
        emit_model(tc, in_aps, out_ap, cfg)
    nc.compile()
    return nc


# ---------------------------------------------------------------- host side

def make_tables(t, n_full):
    # mirror the reference fp32 math
    n = np.arange(n_full, dtype=np.float32)
    q = np.floor(n / 2.0).astype(np.float32) * np.float32(2.0)
    base = np.power(np.float32(THETA), (q / np.float32(n_full)).astype(np.float32))
    freqs = (np.float32(1.0) / base / np.float32(2.0 * np.pi)).astype(np.float32)
    tt = np.arange(t, dtype=np.float32)[:, None]
    phases = (tt * freqs[None, :]).astype(np.float32)
    ph = ((phases % np.float32(1.0)) * np.float32(2.0 * np.pi)).astype(np.float32)
    return np.cos(ph).astype(np.float32), np.sin(ph).astype(np.float32)


def make_in_maps(idx, embed, encoder, encoder_v, decoder, lm_head, cfg):
    n_cores = cfg["n_cores"]
    half = cfg["half"]
    t = cfg["t"]
    ph_loc = half // 2
    nh = n_cores // 2
    n_full = half * 2  # per-head latent dim

    idx = np.asarray(idx).astype(np.int64)
    embed = np.asarray(embed, dtype=np.float32)
    enc = np.asarray(encoder, dtype=np.float32)
    enc_v = np.asarray(encoder_v, dtype=np.float32)
    dec = np.asarray(decoder, dtype=np.float32).reshape(nh, n_full, D)
    lm = np.asarray(lm_head, dtype=np.float32)

    x0 = embed[idx[0]].astype(np.float32)               # [t, D]
    cos_f, sin_f = make_tables(t, n_full)               # [t, n_full]
    lm_bf = lm.astype(BF16NP)
    mask = (np.arange(P)[:, None] < np.arange(P)[None, :]).astype(np.float32)

    in_maps = []
    for c in range(n_cores):
        h, j = divmod(c, 2)
        p_glob = j * ph_loc + np.arange(ph_loc)
        cols = np.concatenate([2 * p_glob, 2 * p_glob + 1])
        in_maps.append({
            "x0": x0,
            "w_e": np.ascontiguousarray(enc[h][:, cols]).astype(BF16NP),
            "w_v": np.ascontiguousarray(enc_v[h][:, cols]).astype(BF16NP),
            "dec": np.ascontiguousarray(dec[h][cols, :]).astype(BF16NP),
            "lm": lm_bf,
            "cos_t": np.ascontiguousarray(cos_f[:, 2 * p_glob].T).astype(BF16NP),
            "sin_t": np.ascontiguousarray(sin_f[:, 2 * p_glob].T).astype(BF16NP),
            "mask": mask,
        })
    return in_maps


def _get_nc(cfg_key=None, cfg=None):
    if cfg is None:
        cfg = _cfg()
    key = tuple(sorted(cfg.items()))
    if key not in _CACHE:
        _CACHE[key] = build(cfg)
    return _CACHE[key]


def run(inputs, cfg=None, trace=False, **run_kwargs):
    from concourse.bass_utils import run_bass_kernel_spmd
    if cfg is None:
        cfg = _cfg()
    nc = _get_nc(cfg=cfg)
    in_maps = make_in_maps(inputs["idx"], inputs["embed"], inputs["encoder"],
                           inputs["encoder_v"], inputs["decoder"],
                           inputs["lm_head"], cfg)
    res = run_bass_kernel_spmd(nc, in_maps, core_ids=list(range(cfg["n_cores"])),
                               trace=trace, **run_kwargs)
    logits = np.asarray(res.results[0]["logits"], dtype=np.float32)
    return logits.reshape(1, cfg["t"], V), res


def kernel(idx, embed, encoder, encoder_v, decoder, lm_head):
    logits, _ = run(dict(idx=idx, embed=embed, encoder=encoder,
                         encoder_v=encoder_v, decoder=decoder, lm_head=lm_head))
    return logits
